# revision 5
# baseline (speedup 1.0000x reference)
"""Adaptive embedding lookup (nn.AdaptiveEmbedding) on 8 TRN2 NeuronCores.

Strategy (data-parallel over tokens, tables replicated, no collectives):

Host:
  - Bucket the 16384 tokens by embedding cluster (4 clusters; cluster 2 is
    further split into 5 sub-ranges of 32000 rows so dma_gather's int16
    indices stay in range; cluster 3's table is packed 8-rows-per-256B-
    super-row), deal each bucket's tokens round-robin to the 8 cores, pad
    each per-core bucket to a multiple of 128 (one PE tile = one output
    "group" of 128 tokens).
  - ALL tables bf16 with 256B-multiple rows so every gather can use
    transpose=True (rows land pre-transposed across partitions — no PE
    transposes, no casts on device): emb0 [20000,1024], emb1 [20000,256],
    emb2 zero-padded to [160000,128], emb3 packed [8467,128].
  - Projections pre-transposed, pre-scaled by sqrt(d_proj), bf16,
    chunk-major; pt2/pt3s/c3-masks packed into one "cst" param to cut
    HWDGE issue count.

Device (SPMD, identical graph on all 8 cores, one TileContext):
  - idx16 DMA first; PE warmup matmuls (HAM clock ramp) run while the
    gathers are in flight; pt0 split in two DMAs so chunk-0 matmuls can
    start as soon as the first c0 gather lands.
  - One transpose dma_gather per bucket pulls rows into SBUF as
    [128 dims, chunk, token] lhsT tiles; per 128-token group: matmul
    against projection chunks accumulating in PSUM (c3 groups first get a
    [128,128] mask-multiply selecting the sub-row inside the packed
    super-row), copy (f32->bf16) to SBUF on vector/scalar, DMA the
    [rows, 1024] output (trimmed to real rows) out on sync.
  - PE stream kept dense and ordered via no-sync scheduling edges.

Host: inverse-permute the 8 per-core outputs into [8, 2048, 1024] f32.
"""

import numpy as np
import ml_dtypes

import concourse.bacc as bacc
import concourse.bass as bass
import concourse.mybir as mybir
from concourse.bass_utils import run_bass_kernel_spmd
from concourse.tile import TileContext

N_TOKEN = 267735
D_PROJ = 1024
CUTOFF_ENDS = [0, 20000, 40000, 200000, 267735]
D_EMBS = [1024, 256, 64, 16]
EMB_SCALE = float(D_PROJ) ** 0.5
N_CORES = 8
P = 128
NFREE = 512          # psum free-dim per matmul
C2_SUB = 32000       # cluster-2 subtable rows (int16 range)
C2_NSUB = 5
C3_PACK = 8          # cluster-3 rows packed per super-row
C3_SROWS = -(-(CUTOFF_ENDS[4] - CUTOFF_ENDS[3]) // C3_PACK)  # 8467

BF16 = ml_dtypes.bfloat16

# Test-harness knobs (the grader never touches these).
TRACE = False
TRACE_CORES = None
LAST = {}

_GRAPH_CACHE = {}

# unit = gather bucket: 0, 1, (2, r) for sub-range r, 3.
UNIT_KEYS = [0, 1] + [(2, r) for r in range(C2_NSUB)] + [3]

N_WARMUP = 18        # PE warmup matmuls (HAM ramp) while gathers fly


def _build_graph(Ks, rows_g):
    """Ks: dict unit_key -> group count (0 allowed); rows_g: global group ->
    output rows actually used (<=128, pad rows trimmed from the out DMA).
    Same on all cores."""
    key = (tuple(Ks[u] for u in UNIT_KEYS), tuple(sorted(rows_g.items())))
    if key in _GRAPH_CACHE:
        return _GRAPH_CACHE[key]

    K0, K1, K3 = Ks[0], Ks[1], Ks[3]
    K2 = sum(Ks[(2, r)] for r in range(C2_NSUB))
    NI = 8 * sum(Ks.values())          # idx16 columns (8 per group)
    G = sum(Ks.values())               # total output groups
    # cst param: [c3 masks | pt2 (padded to 128 part) | pt3s]
    NCST = max(K3, 1) * P + D_PROJ + D_PROJ

    nc = bacc.Bacc("TRN2", debug=False, num_swdge_queues=4)
    idx_ext = nc.declare_dram_parameter("idx16", [P, max(NI, 16)], mybir.dt.int16, False)
    emb0_ext = nc.declare_dram_parameter("emb0b", [20000, 1024], mybir.dt.bfloat16, False)
    emb1_ext = nc.declare_dram_parameter("emb1b", [20000, 256], mybir.dt.bfloat16, False)
    emb2_ext = nc.declare_dram_parameter("emb2p", [C2_SUB * C2_NSUB, 128], mybir.dt.bfloat16, False)
    emb3_ext = nc.declare_dram_parameter("emb3p", [C3_SROWS, 128], mybir.dt.bfloat16, False)
    pt0_ext = nc.declare_dram_parameter("pt0", [128, 8, D_PROJ], mybir.dt.bfloat16, False)
    pt1_ext = nc.declare_dram_parameter("pt1", [128, 2, D_PROJ], mybir.dt.bfloat16, False)
    cst_ext = nc.declare_dram_parameter("cst", [P, NCST], mybir.dt.bfloat16, False)
    out_ext = nc.declare_dram_parameter("out", [G * P, D_PROJ], mybir.dt.bfloat16, True)

    import bass_rust as _br

    with TileContext(nc) as tc:
        with tc.tile_pool(name="const", bufs=1) as constp, \
             tc.tile_pool(name="work", bufs=4) as workp, \
             tc.tile_pool(name="ps_o", bufs=6, space="PSUM") as psump, \
             tc.tile_pool(name="ps_w", bufs=1, space="PSUM") as psumw:
            # idx first: everything downstream (all gathers) waits on it
            idx_sb = constp.tile([P, max(NI, 16)], mybir.dt.int16, tag="idx")
            nc.sync.dma_start(out=idx_sb[:], in_=idx_ext[:])

            # PE warmup on a memset tile — ramps the HAM clock gate to 8/8
            # while the gathers are still in flight
            wtile = constp.tile([P, P], mybir.dt.bfloat16, tag="wm")
            nc.vector.memset(wtile[:], 1.0)
            wps = psumw.tile([P, P], mybir.dt.float32, tag="wps")
            last_pe_inst = [None]
            for _ in range(N_WARMUP):
                mm = nc.tensor.matmul(
                    out=wps[:], lhsT=wtile[:], rhs=wtile[:],
                    start=True, stop=True,
                )
                last_pe_inst[0] = mm.ins

            # gathered lhsT tiles (all land pre-transposed: [dim, ..., token])
            e0 = constp.tile([P, max(K0, 1), 8, P], mybir.dt.bfloat16, tag="e0")
            e1 = constp.tile([P, 2, max(K1, 1) * P], mybir.dt.bfloat16, tag="e1")
            e2s = [
                constp.tile([P, 1, max(Ks[(2, r)], 1) * P], mybir.dt.bfloat16,
                            tag=f"e2_{r}", name=f"e2_{r}")
                for r in range(C2_NSUB)
            ]
            e3 = constp.tile([P, 1, max(K3, 1) * P], mybir.dt.bfloat16, tag="e3")
            em = constp.tile([P, max(K3, 1) * P], mybir.dt.bfloat16, tag="em")

            # idx16 column offset per unit (indices packed in UNIT_KEYS order)
            unit_col = {}
            col = 0
            for u in UNIT_KEYS:
                unit_col[u] = col
                col += 8 * Ks[u]

            # gather emission: c0 per group first (matmuls start as soon as
            # group 0 lands), then c1, c2 sub-ranges, c3; spread over queues
            qi = 0

            def gather(dst, tab, u, glo, n, elem):
                nonlocal qi
                c0_ = unit_col[u] + 8 * glo
                nc.gpsimd.dma_gather(
                    dst, tab, idx_sb[:, c0_:c0_ + 8 * n], n * P, n * P, elem,
                    transpose=True,
                    queue_num=qi % 4,
                )
                qi += 1

            for glo in range(K0):
                gather(e0[:, glo, :, :], emb0_ext[:], 0, glo, 1, 1024)
            if K1 > 0:
                gather(e1[:], emb1_ext[:], 1, 0, K1, 256)
            for r in range(C2_NSUB):
                n = Ks[(2, r)]
                if n > 0:
                    gather(
                        e2s[r][:],
                        emb2_ext[r * C2_SUB:(r + 1) * C2_SUB, :],
                        (2, r), 0, n, 128,
                    )
            if K3 > 0:
                gather(e3[:], emb3_ext[:], 3, 0, K3, 128)

            # projections / consts; pt0 split in two so chunk-0..3 matmuls
            # can start before chunks 4..7 land
            pt0_sb = constp.tile([128, 8, D_PROJ], mybir.dt.bfloat16, tag="pt0")
            nc.sync.dma_start(out=pt0_sb[:, 0:4, :], in_=pt0_ext[:, 0:4, :])
            nc.sync.dma_start(out=pt0_sb[:, 4:8, :], in_=pt0_ext[:, 4:8, :])
            pt1_sb = constp.tile([128, 2, D_PROJ], mybir.dt.bfloat16, tag="pt1")
            nc.sync.dma_start(out=pt1_sb[:], in_=pt1_ext[:])
            cst_sb = constp.tile([P, NCST], mybir.dt.bfloat16, tag="cst")
            nc.scalar.dma_start(out=cst_sb[:], in_=cst_ext[:])
            mask_v = cst_sb[:, 0:max(K3, 1) * P]
            pt2_sb = cst_sb[:, max(K3, 1) * P:max(K3, 1) * P + D_PROJ]
            pt3_sb = cst_sb[:, max(K3, 1) * P + D_PROJ:]

            # global group index per (unit, local group), in UNIT_KEYS order
            gbase_dev = {}
            acc_g = 0
            for u in UNIT_KEYS:
                gbase_dev[u] = acc_g
                acc_g += Ks[u]

            cp_ct = [0]  # copy-engine round robin

            def emit_group(ch, lhsT_of, rhs_of, g):
                osb = workp.tile([P, D_PROJ], mybir.dt.bfloat16, tag="osb")
                ps0 = psump.tile([P, NFREE], mybir.dt.float32, tag="ps")
                ps1 = psump.tile([P, NFREE], mybir.dt.float32, tag="ps")
                pss = [ps0, ps1]
                for kc in range(ch):
                    lt = lhsT_of(kc)
                    for oc, ps in enumerate(pss):
                        mm = nc.tensor.matmul(
                            out=ps[:],
                            lhsT=lt,
                            rhs=rhs_of(kc, oc),
                            start=(kc == 0),
                            stop=(kc == ch - 1),
                        )
                        if last_pe_inst[0] is not None:
                            _br.add_dep_helper(
                                mm.ins, last_pe_inst[0], sync=False,
                                reason="keep PE stream in gather-landing order",
                            )
                        last_pe_inst[0] = mm.ins
                for oc, ps in enumerate(pss):
                    dst = osb[:, oc * NFREE:(oc + 1) * NFREE]
                    if cp_ct[0] % 2 == 0:
                        nc.vector.tensor_copy(out=dst, in_=ps[:])
                    else:
                        nc.scalar.copy(out=dst, in_=ps[:])
                    cp_ct[0] += 1
                rows = rows_g[g]
                nc.sync.dma_start(
                    out=out_ext[g * P:g * P + rows, :], in_=osb[:rows, :]
                )

            # ---- c0 (8 contraction chunks) ----
            for j in range(K0):
                emit_group(
                    8,
                    lambda kc, _j=j: e0[:, _j, kc, :],
                    lambda kc, oc: pt0_sb[:, kc, oc * NFREE:(oc + 1) * NFREE],
                    gbase_dev[0] + j,
                )
            # ---- c1 (2 chunks) ----
            for j in range(K1):
                emit_group(
                    2,
                    lambda kc, _j=j: e1[:, kc, _j * P:(_j + 1) * P],
                    lambda kc, oc: pt1_sb[:, kc, oc * NFREE:(oc + 1) * NFREE],
                    gbase_dev[1] + j,
                )
            # ---- c2 (contraction 64, top 64 partitions of the padded rows
            # are unread) ----
            jg = 0
            for r in range(C2_NSUB):
                for j in range(Ks[(2, r)]):
                    emit_group(
                        1,
                        lambda kc, _r=r, _j=j: e2s[_r][:64, 0, _j * P:(_j + 1) * P],
                        lambda kc, oc: pt2_sb[:64, oc * NFREE:(oc + 1) * NFREE],
                        gbase_dev[(2, 0)] + jg,
                    )
                    jg += 1
            # ---- c3: per-group mask-select of the 16-elem sub-row inside
            # the 128-elem packed super-row, then matmul vs the 8x-tiled
            # projection ----
            for j in range(K3):
                nc.vector.tensor_tensor(
                    out=em[:, j * P:(j + 1) * P],
                    in0=e3[:, 0, j * P:(j + 1) * P],
                    in1=mask_v[:, j * P:(j + 1) * P],
                    op=mybir.AluOpType.mult,
                )
                emit_group(
                    1,
                    lambda kc, _j=j: em[:, _j * P:(_j + 1) * P],
                    lambda kc, oc: pt3_sb[:, oc * NFREE:(oc + 1) * NFREE],
                    gbase_dev[3] + j,
                )

    nc.compile()
    _GRAPH_CACHE[key] = nc
    return nc


def _wrap_idx16(vals, n_slots):
    """int16 values (len <= n_slots, padded with 0) -> [128, n_slots/16] wrapped."""
    full = np.zeros(n_slots, dtype=np.int16)
    full[:len(vals)] = vals
    w = np.zeros((16, n_slots // 16), dtype=np.int16)
    m = np.arange(n_slots)
    w[m % 16, m // 16] = full
    return np.tile(w, (8, 1))


def kernel(inp, emb0, emb1, emb2, emb3, proj0, proj1, proj2, proj3):
    inp = np.asarray(inp)
    embs = [np.asarray(e) for e in (emb0, emb1, emb2, emb3)]
    projs = [np.asarray(p) for p in (proj0, proj1, proj2, proj3)]
    B, S = inp.shape
    flat = inp.reshape(-1).astype(np.int64)
    T = flat.shape[0]

    # ---- host-side bucketing -------------------------------------------
    flat = np.clip(flat, 0, N_TOKEN - 1)
    cluster = np.clip(
        np.searchsorted(np.asarray(CUTOFF_ENDS[1:]), flat, side="right"), 0, 3
    )
    local = flat - np.asarray(CUTOFF_ENDS)[cluster]

    unit_pos = {}
    for u in UNIT_KEYS:
        if u == 0 or u == 1 or u == 3:
            unit_pos[u] = np.nonzero(cluster == u)[0]
        else:
            r = u[1]
            unit_pos[u] = np.nonzero((cluster == 2) & (local // C2_SUB == r))[0]

    core_lists = {u: [unit_pos[u][k::N_CORES] for k in range(N_CORES)]
                  for u in UNIT_KEYS}
    Ks = {
        u: int(-(-max(len(core_lists[u][k]) for k in range(N_CORES)) // P))
        for u in UNIT_KEYS
    }
    G = sum(Ks.values())
    K3 = Ks[3]

    def idxval(u, positions):
        lv = local[positions]
        if u == 0 or u == 1:
            return lv.astype(np.int16)
        if u == 3:
            return (lv // C3_PACK).astype(np.int16)
        return (lv - u[1] * C2_SUB).astype(np.int16)

    NI = 8 * G
    gbase = {}
    acc = 0
    for u in UNIT_KEYS:
        gbase[u] = acc
        acc += Ks[u]

    NCST = max(K3, 1) * P + D_PROJ + D_PROJ
    blkid = np.arange(128) // 16  # sub-row block of each super-row element

    # pt2 (padded to 128 partitions) and pt3s are core-independent
    pt2pad = np.zeros((P, D_PROJ), dtype=np.float32)
    pt2pad[:64] = projs[2].T.astype(np.float32) * EMB_SCALE
    pt3s = np.tile(projs[3].T.astype(np.float32) * EMB_SCALE, (C3_PACK, 1))

    idx_maps, cst_maps, row_maps = [], [], []
    for k in range(N_CORES):
        cols = []
        row_map = np.full(G * P, -1, dtype=np.int64)
        cst = np.zeros((P, NCST), dtype=np.float32)
        cst[:, max(K3, 1) * P:max(K3, 1) * P + D_PROJ] = pt2pad
        cst[:, max(K3, 1) * P + D_PROJ:] = pt3s
        for u in UNIT_KEYS:
            n = Ks[u]
            if n == 0:
                continue
            lst = core_lists[u][k]
            cols.append(_wrap_idx16(idxval(u, lst), n * P))
            m = np.arange(len(lst))
            row_map[(gbase[u] + m // P) * P + (m % P)] = lst
            if u == 3:
                # transposed mask layout: [dim-partition, token col]
                s_arr = local[lst] % C3_PACK                 # [n_tok]
                mask = np.zeros((P, K3 * P), dtype=np.float32)
                mask[:, m] = (blkid[:, None] == s_arr[None, :])
                cst[:, 0:K3 * P] = mask
        idx_host = (np.concatenate(cols, axis=1) if cols
                    else np.zeros((P, 16), np.int16))
        if idx_host.shape[1] < max(NI, 16):
            pad = np.zeros((P, max(NI, 16) - idx_host.shape[1]), np.int16)
            idx_host = np.concatenate([idx_host, pad], axis=1)
        idx_maps.append(np.ascontiguousarray(idx_host))
        cst_maps.append(cst.astype(BF16))
        row_maps.append(row_map)

    # ---- table/projection prep -----------------------------------------
    emb0b = np.ascontiguousarray(embs[0].astype(BF16))
    emb1b = np.ascontiguousarray(embs[1].astype(BF16))
    emb2p = np.zeros((C2_SUB * C2_NSUB, 128), dtype=BF16)
    emb2p[:160000, :64] = embs[2].astype(BF16)
    e3flat = embs[3].astype(np.float32)
    pad3 = C3_SROWS * C3_PACK - e3flat.shape[0]
    e3flat = np.concatenate([e3flat, np.zeros((pad3, 16), np.float32)], axis=0)
    emb3p = np.ascontiguousarray(e3flat.reshape(C3_SROWS, 128).astype(BF16))

    pts = {}
    for c, name, pc, ch in ((0, "pt0", 128, 8), (1, "pt1", 128, 2)):
        ptc = (projs[c].T.astype(np.float32) * EMB_SCALE).astype(BF16)
        pts[name] = np.ascontiguousarray(
            ptc.reshape(ch, pc, D_PROJ).transpose(1, 0, 2)
        )

    in_maps = []
    for k in range(N_CORES):
        m = {
            "idx16": idx_maps[k], "cst": cst_maps[k],
            "emb0b": emb0b, "emb1b": emb1b, "emb2p": emb2p, "emb3p": emb3p,
        }
        m.update(pts)
        in_maps.append(m)

    # ---- device --------------------------------------------------------
    rows_g = {}
    for u in UNIT_KEYS:
        maxcnt = max(len(core_lists[u][k]) for k in range(N_CORES))
        for t in range(Ks[u]):
            rows_g[gbase[u] + t] = int(min(P, max(1, maxcnt - t * P)))
    nc = _build_graph(Ks, rows_g)
    res = run_bass_kernel_spmd(
        nc,
        in_maps,
        core_ids=list(range(N_CORES)),
        trace=TRACE,
        trace_cores=TRACE_CORES,
    )
    LAST["res"] = res
    LAST["Ks"] = Ks

    # ---- host-side unshard ---------------------------------------------
    out_full = np.zeros((T, D_PROJ), dtype=np.float32)
    for k in range(N_CORES):
        o = np.asarray(res.results[k]["out"])
        rm = row_maps[k]
        valid = rm >= 0
        out_full[rm[valid]] = o[valid].astype(np.float32)
    return out_full.reshape(B, S, D_PROJ)


# revision 7
# speedup vs baseline: 1.2038x; 1.2038x over previous
"""Adaptive embedding lookup (nn.AdaptiveEmbedding) on 8 TRN2 NeuronCores.

Strategy (data-parallel over tokens, tables replicated, no collectives):

Host:
  - Bucket the 16384 tokens by embedding cluster (4 clusters; cluster 2 is
    further split into 5 sub-ranges of 32000 rows so dma_gather's int16
    indices stay in range; cluster 3's table is packed 8-rows-per-256B-
    super-row), deal each bucket's tokens round-robin to the 8 cores, pad
    each per-core bucket to a multiple of 128 (one PE tile = one output
    "group" of 128 tokens).
  - ALL tables bf16 with 256B-multiple rows so every gather can use
    transpose=True (rows land pre-transposed across partitions — no PE
    transposes, no casts on device): emb0 [20000,1024], emb1 [20000,256],
    emb2 zero-padded to [160000,128], emb3 packed [8467,128].
  - Projections pre-transposed, pre-scaled by sqrt(d_proj), bf16,
    chunk-major; pt2/pt3s/c3-masks packed into one "cst" param to cut
    HWDGE issue count.

Device (SPMD, identical graph on all 8 cores, one TileContext):
  - The dma_gather ucode library load blocks all gathers until ~21us in;
    a long PE warmup burst (HAM clock ramp) and the const loads fill that
    window.  idx16 DMA goes first; pt0 is split in two DMAs.
  - Per-group transpose dma_gathers (round-robin over the 4 SWDGE queues)
    pull rows into SBUF as [dim, token] lhsT tiles; per 128-token group:
    matmul against projection chunks accumulating into a 2-bank PSUM tile
    (c3 groups first get a [128,128] mask-multiply selecting the sub-row
    of the packed super-row), then ONE [128,1024] f32->bf16 copy into a
    per-unit staging tile (vector/scalar alternating).
  - One batched out DMA per unit ([128, K, 1024] SBUF -> [K*128, 1024]
    rows in HBM, sync/scalar alternating) instead of 19 per-group DMAs —
    the per-issue cost (~0.8us) and completion handling dominated v1.

Host: inverse-permute the 8 per-core outputs into [8, 2048, 1024] f32.
"""

import numpy as np
import ml_dtypes

import concourse.bacc as bacc
import concourse.bass as bass
import concourse.mybir as mybir
from concourse.bass_utils import run_bass_kernel_spmd
from concourse.tile import TileContext

N_TOKEN = 267735
D_PROJ = 1024
CUTOFF_ENDS = [0, 20000, 40000, 200000, 267735]
D_EMBS = [1024, 256, 64, 16]
EMB_SCALE = float(D_PROJ) ** 0.5
N_CORES = 8
P = 128
NFREE = 512          # psum free-dim per matmul
C2_SUB = 32000       # cluster-2 subtable rows (int16 range)
C2_NSUB = 5
C3_PACK = 8          # cluster-3 rows packed per super-row
C3_SROWS = -(-(CUTOFF_ENDS[4] - CUTOFF_ENDS[3]) // C3_PACK)  # 8467

BF16 = ml_dtypes.bfloat16

# Test-harness knobs (the grader never touches these).
TRACE = False
TRACE_CORES = None
LAST = {}

_GRAPH_CACHE = {}

# unit = gather bucket: 0, 1, (2, r) for sub-range r, 3.
UNIT_KEYS = [0, 1] + [(2, r) for r in range(C2_NSUB)] + [3]

N_WARMUP = 96        # PE warmup matmuls (HAM ramp) while the ucode lib loads


def _build_graph(Ks):
    """Ks: dict unit_key -> group count (0 allowed). Same on all cores."""
    key = tuple(Ks[u] for u in UNIT_KEYS)
    if key in _GRAPH_CACHE:
        return _GRAPH_CACHE[key]

    K0, K1, K3 = Ks[0], Ks[1], Ks[3]
    K2 = sum(Ks[(2, r)] for r in range(C2_NSUB))
    NI = 8 * sum(Ks.values())          # idx16 columns (8 per group)
    G = sum(Ks.values())               # total output groups
    # cst param: [c3 masks | pt2 (padded to 128 part) | pt3s]
    NCST = max(K3, 1) * P + D_PROJ + D_PROJ

    nc = bacc.Bacc("TRN2", debug=False, num_swdge_queues=4)
    idx_ext = nc.declare_dram_parameter("idx16", [P, max(NI, 16)], mybir.dt.int16, False)
    emb0_ext = nc.declare_dram_parameter("emb0b", [20000, 1024], mybir.dt.bfloat16, False)
    emb1_ext = nc.declare_dram_parameter("emb1b", [20000, 256], mybir.dt.bfloat16, False)
    emb2_ext = nc.declare_dram_parameter("emb2p", [C2_SUB * C2_NSUB, 128], mybir.dt.bfloat16, False)
    emb3_ext = nc.declare_dram_parameter("emb3p", [C3_SROWS, 128], mybir.dt.bfloat16, False)
    pt0_ext = nc.declare_dram_parameter("pt0", [128, 8, D_PROJ], mybir.dt.bfloat16, False)
    pt1_ext = nc.declare_dram_parameter("pt1", [128, 2, D_PROJ], mybir.dt.bfloat16, False)
    cst_ext = nc.declare_dram_parameter("cst", [P, NCST], mybir.dt.bfloat16, False)
    out_ext = nc.declare_dram_parameter("out", [G * P, D_PROJ], mybir.dt.bfloat16, True)

    import bass_rust as _br

    with TileContext(nc) as tc:
        with tc.tile_pool(name="const", bufs=1) as constp, \
             tc.tile_pool(name="ps_o", bufs=3, space="PSUM") as psump, \
             tc.tile_pool(name="ps_w", bufs=1, space="PSUM") as psumw:
            # idx first: everything downstream (all gathers) waits on it
            idx_sb = constp.tile([P, max(NI, 16)], mybir.dt.int16, tag="idx")
            nc.sync.dma_start(out=idx_sb[:], in_=idx_ext[:])

            # PE warmup on a memset tile — ramps the HAM clock gate to 8/8
            # and keeps it there until the gathers (blocked behind the
            # gpsimd ucode library load) can feed real matmuls
            wtile = constp.tile([P, P], mybir.dt.bfloat16, tag="wm")
            nc.vector.memset(wtile[:], 1.0)
            wps = psumw.tile([P, P], mybir.dt.float32, tag="wps")
            last_pe_inst = [None]
            for _ in range(N_WARMUP):
                mm = nc.tensor.matmul(
                    out=wps[:], lhsT=wtile[:], rhs=wtile[:],
                    start=True, stop=True,
                )
                last_pe_inst[0] = mm.ins

            # gathered lhsT tiles (all land pre-transposed: [dim, ..., token])
            e0 = constp.tile([P, max(K0, 1), 8, P], mybir.dt.bfloat16, tag="e0")
            e1 = constp.tile([P, max(K1, 1), 2, P], mybir.dt.bfloat16, tag="e1")
            e2s = [
                constp.tile([P, max(Ks[(2, r)], 1), 1, P], mybir.dt.bfloat16,
                            tag=f"e2_{r}", name=f"e2_{r}")
                for r in range(C2_NSUB)
            ]
            e3 = constp.tile([P, max(K3, 1), 1, P], mybir.dt.bfloat16, tag="e3")
            em = constp.tile([P, max(K3, 1) * P], mybir.dt.bfloat16, tag="em")

            # idx16 column offset per unit (indices packed in UNIT_KEYS order)
            unit_col = {}
            col = 0
            for u in UNIT_KEYS:
                unit_col[u] = col
                col += 8 * Ks[u]

            # per-group gathers, spread over the 4 SWDGE queues in
            # PE-consumption order so data lands in the order it is needed
            def gather(dst, tab, u, glo, qn, elem):
                c0_ = unit_col[u] + 8 * glo
                nc.gpsimd.dma_gather(
                    dst, tab, idx_sb[:, c0_:c0_ + 8], P, P, elem,
                    transpose=True,
                    queue_num=qn,
                )

            qn = 0
            for glo in range(K0):
                gather(e0[:, glo, :, :], emb0_ext[:], 0, glo, qn % 4, 1024)
                qn += 1
            for glo in range(K1):
                gather(e1[:, glo, :, :], emb1_ext[:], 1, glo, qn % 4, 256)
                qn += 1
            c2glo = []
            for r in range(C2_NSUB):
                for j in range(Ks[(2, r)]):
                    c2glo.append((r, j))
            for r, j in c2glo:
                gather(e2s[r][:, j, :, :],
                       emb2_ext[r * C2_SUB:(r + 1) * C2_SUB, :],
                       (2, r), j, qn % 4, 128)
                qn += 1
            for glo in range(K3):
                gather(e3[:, glo, :, :], emb3_ext[:], 3, glo, qn % 4, 128)
                qn += 1

            # projections / consts; pt0 split in two so chunk-0..3 matmuls
            # can start before chunks 4..7 land
            pt0_sb = constp.tile([128, 8, D_PROJ], mybir.dt.bfloat16, tag="pt0")
            nc.sync.dma_start(out=pt0_sb[:, 0:4, :], in_=pt0_ext[:, 0:4, :])
            nc.sync.dma_start(out=pt0_sb[:, 4:8, :], in_=pt0_ext[:, 4:8, :])
            pt1_sb = constp.tile([128, 2, D_PROJ], mybir.dt.bfloat16, tag="pt1")
            nc.sync.dma_start(out=pt1_sb[:], in_=pt1_ext[:])
            cst_sb = constp.tile([P, NCST], mybir.dt.bfloat16, tag="cst")
            nc.scalar.dma_start(out=cst_sb[:], in_=cst_ext[:])
            mask_v = cst_sb[:, 0:max(K3, 1) * P]
            pt2_sb = cst_sb[:, max(K3, 1) * P:max(K3, 1) * P + D_PROJ]
            pt3_sb = cst_sb[:, max(K3, 1) * P + D_PROJ:]

            # per-unit staging tiles for the batched out DMAs
            stage = {}
            for u in UNIT_KEYS:
                stage[u] = constp.tile(
                    [P, max(Ks[u], 1), D_PROJ], mybir.dt.bfloat16,
                    tag=f"st{u}", name=f"st_{u}",
                )

            # global group base per unit (defines out rows), UNIT_KEYS order
            gbase_dev = {}
            acc_g = 0
            for u in UNIT_KEYS:
                gbase_dev[u] = acc_g
                acc_g += Ks[u]

            cp_ct = [0]   # copy-engine round robin
            out_ct = [0]  # out-DMA engine round robin

            def emit_group(ch, lhsT_of, rhs_of, st, j):
                ps = psump.tile([P, 2 * NFREE], mybir.dt.float32, tag="ps")
                for kc in range(ch):
                    lt = lhsT_of(kc)
                    for oc in range(2):
                        mm = nc.tensor.matmul(
                            out=ps[:, oc * NFREE:(oc + 1) * NFREE],
                            lhsT=lt,
                            rhs=rhs_of(kc, oc),
                            start=(kc == 0),
                            stop=(kc == ch - 1),
                        )
                        if last_pe_inst[0] is not None:
                            _br.add_dep_helper(
                                mm.ins, last_pe_inst[0], sync=False,
                                reason="keep PE stream in gather-landing order",
                            )
                        last_pe_inst[0] = mm.ins
                dst = st[:, j, :]
                if cp_ct[0] % 2 == 0:
                    nc.vector.tensor_copy(out=dst, in_=ps[:])
                else:
                    nc.scalar.copy(out=dst, in_=ps[:])
                cp_ct[0] += 1

            def emit_unit_out(u):
                K = Ks[u]
                if K == 0:
                    return
                base = gbase_dev[u] * P
                dst = out_ext[base:base + K * P, :].rearrange(
                    "(q p) d -> p q d", p=P
                )
                eng = nc.sync if out_ct[0] % 2 == 0 else nc.scalar
                out_ct[0] += 1
                eng.dma_start(out=dst, in_=stage[u][:])

            # ---- c0 (8 contraction chunks) ----
            for j in range(K0):
                emit_group(
                    8,
                    lambda kc, _j=j: e0[:, _j, kc, :],
                    lambda kc, oc: pt0_sb[:, kc, oc * NFREE:(oc + 1) * NFREE],
                    stage[0], j,
                )
            emit_unit_out(0)
            # ---- c1 (2 chunks) ----
            for j in range(K1):
                emit_group(
                    2,
                    lambda kc, _j=j: e1[:, _j, kc, :],
                    lambda kc, oc: pt1_sb[:, kc, oc * NFREE:(oc + 1) * NFREE],
                    stage[1], j,
                )
            emit_unit_out(1)
            # ---- c2 (contraction 64, top 64 partitions of the padded rows
            # are unread) ----
            for r in range(C2_NSUB):
                for j in range(Ks[(2, r)]):
                    emit_group(
                        1,
                        lambda kc, _r=r, _j=j: e2s[_r][:64, _j, 0, :],
                        lambda kc, oc: pt2_sb[:64, oc * NFREE:(oc + 1) * NFREE],
                        stage[(2, r)], j,
                    )
                emit_unit_out((2, r))
            # ---- c3: per-group mask-select of the 16-elem sub-row inside
            # the 128-elem packed super-row, then matmul vs the 8x-tiled
            # projection ----
            for j in range(K3):
                nc.vector.tensor_tensor(
                    out=em[:, j * P:(j + 1) * P],
                    in0=e3[:, j, 0, :],
                    in1=mask_v[:, j * P:(j + 1) * P],
                    op=mybir.AluOpType.mult,
                )
                emit_group(
                    1,
                    lambda kc, _j=j: em[:, _j * P:(_j + 1) * P],
                    lambda kc, oc: pt3_sb[:, oc * NFREE:(oc + 1) * NFREE],
                    stage[3], j,
                )
            emit_unit_out(3)

    nc.compile()
    _GRAPH_CACHE[key] = nc
    return nc


def _wrap_idx16(vals, n_slots):
    """int16 values (len <= n_slots, padded with 0) -> [128, n_slots/16] wrapped."""
    full = np.zeros(n_slots, dtype=np.int16)
    full[:len(vals)] = vals
    w = np.zeros((16, n_slots // 16), dtype=np.int16)
    m = np.arange(n_slots)
    w[m % 16, m // 16] = full
    return np.tile(w, (8, 1))


def kernel(inp, emb0, emb1, emb2, emb3, proj0, proj1, proj2, proj3):
    inp = np.asarray(inp)
    embs = [np.asarray(e) for e in (emb0, emb1, emb2, emb3)]
    projs = [np.asarray(p) for p in (proj0, proj1, proj2, proj3)]
    B, S = inp.shape
    flat = inp.reshape(-1).astype(np.int64)
    T = flat.shape[0]

    # ---- host-side bucketing -------------------------------------------
    flat = np.clip(flat, 0, N_TOKEN - 1)
    cluster = np.clip(
        np.searchsorted(np.asarray(CUTOFF_ENDS[1:]), flat, side="right"), 0, 3
    )
    local = flat - np.asarray(CUTOFF_ENDS)[cluster]

    unit_pos = {}
    for u in UNIT_KEYS:
        if u == 0 or u == 1 or u == 3:
            unit_pos[u] = np.nonzero(cluster == u)[0]
        else:
            r = u[1]
            unit_pos[u] = np.nonzero((cluster == 2) & (local // C2_SUB == r))[0]

    core_lists = {u: [unit_pos[u][k::N_CORES] for k in range(N_CORES)]
                  for u in UNIT_KEYS}
    Ks = {
        u: int(-(-max(len(core_lists[u][k]) for k in range(N_CORES)) // P))
        for u in UNIT_KEYS
    }
    G = sum(Ks.values())
    K3 = Ks[3]

    def idxval(u, positions):
        lv = local[positions]
        if u == 0 or u == 1:
            return lv.astype(np.int16)
        if u == 3:
            return (lv // C3_PACK).astype(np.int16)
        return (lv - u[1] * C2_SUB).astype(np.int16)

    NI = 8 * G
    gbase = {}
    acc = 0
    for u in UNIT_KEYS:
        gbase[u] = acc
        acc += Ks[u]

    NCST = max(K3, 1) * P + D_PROJ + D_PROJ
    blkid = np.arange(128) // 16  # sub-row block of each super-row element

    # pt2 (padded to 128 partitions) and pt3s are core-independent
    pt2pad = np.zeros((P, D_PROJ), dtype=np.float32)
    pt2pad[:64] = projs[2].T.astype(np.float32) * EMB_SCALE
    pt3s = np.tile(projs[3].T.astype(np.float32) * EMB_SCALE, (C3_PACK, 1))

    idx_maps, cst_maps, row_maps = [], [], []
    for k in range(N_CORES):
        cols = []
        row_map = np.full(G * P, -1, dtype=np.int64)
        cst = np.zeros((P, NCST), dtype=np.float32)
        cst[:, max(K3, 1) * P:max(K3, 1) * P + D_PROJ] = pt2pad
        cst[:, max(K3, 1) * P + D_PROJ:] = pt3s
        for u in UNIT_KEYS:
            n = Ks[u]
            if n == 0:
                continue
            lst = core_lists[u][k]
            cols.append(_wrap_idx16(idxval(u, lst), n * P))
            m = np.arange(len(lst))
            row_map[(gbase[u] + m // P) * P + (m % P)] = lst
            if u == 3:
                # transposed mask layout: [dim-partition, token col]
                s_arr = local[lst] % C3_PACK                 # [n_tok]
                mask = np.zeros((P, K3 * P), dtype=np.float32)
                mask[:, m] = (blkid[:, None] == s_arr[None, :])
                cst[:, 0:K3 * P] = mask
        idx_host = (np.concatenate(cols, axis=1) if cols
                    else np.zeros((P, 16), np.int16))
        if idx_host.shape[1] < max(NI, 16):
            pad = np.zeros((P, max(NI, 16) - idx_host.shape[1]), np.int16)
            idx_host = np.concatenate([idx_host, pad], axis=1)
        idx_maps.append(np.ascontiguousarray(idx_host))
        cst_maps.append(cst.astype(BF16))
        row_maps.append(row_map)

    # ---- table/projection prep -----------------------------------------
    emb0b = np.ascontiguousarray(embs[0].astype(BF16))
    emb1b = np.ascontiguousarray(embs[1].astype(BF16))
    emb2p = np.zeros((C2_SUB * C2_NSUB, 128), dtype=BF16)
    emb2p[:160000, :64] = embs[2].astype(BF16)
    e3flat = embs[3].astype(np.float32)
    pad3 = C3_SROWS * C3_PACK - e3flat.shape[0]
    e3flat = np.concatenate([e3flat, np.zeros((pad3, 16), np.float32)], axis=0)
    emb3p = np.ascontiguousarray(e3flat.reshape(C3_SROWS, 128).astype(BF16))

    pts = {}
    for c, name, pc, ch in ((0, "pt0", 128, 8), (1, "pt1", 128, 2)):
        ptc = (projs[c].T.astype(np.float32) * EMB_SCALE).astype(BF16)
        pts[name] = np.ascontiguousarray(
            ptc.reshape(ch, pc, D_PROJ).transpose(1, 0, 2)
        )

    in_maps = []
    for k in range(N_CORES):
        m = {
            "idx16": idx_maps[k], "cst": cst_maps[k],
            "emb0b": emb0b, "emb1b": emb1b, "emb2p": emb2p, "emb3p": emb3p,
        }
        m.update(pts)
        in_maps.append(m)

    # ---- device --------------------------------------------------------
    nc = _build_graph(Ks)
    res = run_bass_kernel_spmd(
        nc,
        in_maps,
        core_ids=list(range(N_CORES)),
        trace=TRACE,
        trace_cores=TRACE_CORES,
    )
    LAST["res"] = res
    LAST["Ks"] = Ks

    # ---- host-side unshard ---------------------------------------------
    out_full = np.zeros((T, D_PROJ), dtype=np.float32)
    for k in range(N_CORES):
        o = np.asarray(res.results[k]["out"])
        rm = row_maps[k]
        valid = rm >= 0
        out_full[rm[valid]] = o[valid].astype(np.float32)
    return out_full.reshape(B, S, D_PROJ)
